# revision 27
# baseline (speedup 1.0000x reference)
"""Trainium2 Bass kernel for nn_DividedSsimLoss.

Reference: for 8 RGB 1024x1024 image pairs, grayscale, tile 256x256,
9-level 2x2 sum-pool pyramid, loss = sum_d K[d] * (1 - mean ssim_d),
ssim = (2st + C1) / (s^2 + t^2 + C1), i.e. 1-ssim = (s-t)^2/(s^2+t^2+C1).

v4 design (per core = one image pair, pure data parallelism):
  * Host ships the 6 channel planes as fp8e4m3, with R and B pre-scaled
    by their grayscale weights (wr/wg, wb/wg) so that every device-side
    channel-mix weight is exactly +-1.0 in fp8.  Layout per partition
    line: [x|y][R',B',G][1024] -> one [128, 6144] fp8 DMA per 128-row
    chunk (6 KiB contiguous DRAM per partition line).  6 MiB per core.
  * Grayscale (scaled 1/wg) runs on the tensor engine with fp8
    DoubleRow matmuls: one DR matmul contracts the stacked [R'; B']
    2-k-tile pair at 0.5 cyc/row, a second DR matmul adds G via a
    [I; 0] / [0; I] weight pair.  PSUM accumulates in f32; the scalar
    engine evacuates to SBUF as f32r-rounded f32.
  * diff = gray_x - gray_y is ALSO computed on the tensor engine
    ([I;I] / [-I;-I] / [I;-I] DR matmuls into separate PSUM slabs),
    exactly consistent with s - t.
  * Per level the vector engine runs only 2 custom DVE passes:
    DEN = s^2+t^2+C1 (from SBUF) and the fused
    RCPMUL: accum += diff^2 * recip_approx(den), with diff read
    directly from PSUM (one PSUM operand is allowed).
  * 2x2 pooling: row pairs via Pa/Pb f32r matmuls with stride-2 rhs
    views (column pairs fold into the same PSUM accumulation), scalar
    engine evacuates.  Levels 7/6 pyramid diffs via +-I f32r matmuls.
  * Device covers levels 8,7,6 + pooled level-5 images; host does
    levels 5..0 in f64.
"""

import os
import sys

import numpy as np

for _p in ("/opt/trn_rl_repo",):
    if _p not in sys.path:
        sys.path.insert(0, _p)

import concourse.bacc as bacc
import concourse.bass as bass
import concourse.mybir as mybir
import concourse.tile as tile
from concourse.bass_utils import run_bass_kernel_spmd

from ml_dtypes import bfloat16 as np_bf16


def _register_dve_ops():
    """Register kernel-specific custom DVE ops (idempotent).

    DEN_SSIM:  out = in0^2 + in1^2 + s0
    RCPMUL:    out = in1^2 * y1(in0),  accum = sum(out)
               y1 = one-NR reciprocal approx of in0 (bitwise-not seed)
    """
    import concourse.dve_ops as dve_ops
    from concourse.dve_ops import DveOp
    from concourse.dve_spec import (
        C0,
        C1,
        AluOp,
        Bin,
        Spec,
        Src0,
        Src1,
        _has_src1,
        lower,
        sq,
    )
    from concourse.dve_uop import DveOpSpec
    from operator import add as _add

    def _sha_for(name, spec):
        shas = {}
        for ver in ("v3",):
            row = dve_ops._SUB_OPCODE_FOR_NAME[name]
            s = DveOpSpec(
                name=name, opcode=row, uops=lower(spec, ver=ver),
                rd1_en=_has_src1(spec),
            )
            shas[ver] = s.sha(ver)
        return shas

    def _register(name, spec):
        if name in dve_ops._SUB_OPCODE_FOR_NAME:
            return next(op for op in dve_ops.OPS if op.name == name)
        row = dve_ops._CUSTOM_DVE_ROW_BASE + len(dve_ops.OPS)
        assert row < 0x20, "custom-DVE row field overflow"
        dve_ops._SUB_OPCODE_FOR_NAME[name] = row
        op = DveOp(name, spec, subdim=False, uops_sha=_sha_for(name, spec))
        dve_ops.OPS.append(op)
        dve_ops.CUSTOM_DVE_SPECS[name] = spec
        return op

    den_spec = Spec(
        body=sq(Src0) + sq(Src1) + C0,
        reference=lambda in0, in1, s0, s1, imm2: (
            in0.astype(np.float32) ** 2 + in1.astype(np.float32) ** 2 + s0
        ),
    )

    # reciprocal seed: x * bitcast(~x) lands in [-4.5, -4]; one Chebyshev
    # scale + one NR pass (same constants as RECIPROCAL_APPROX_FAST).
    _nx = Bin(AluOp.BITWISE_NOT, Src0, Src0)
    _y0 = _nx * C0
    _y1 = _y0 * (C1 - Src0 * _y0)

    def _ref_rcpmul(in0, in1, c0, c1, c2):
        not_x = (~in0.astype(np.float32).view(np.int32)).view(np.float32)
        y0 = not_x * c0
        y1 = y0 * (c1 - in0.astype(np.float32) * y0)
        return in1.astype(np.float32) ** 2 * y1

    rcpmul_spec = Spec(
        body=sq(Src1) * _y1,
        accum=_add,
        reference=dve_ops._ref_body_sum(_ref_rcpmul),
    )

    return (
        _register("DEN_SSIM_ANT", den_spec),
        _register("RCPMUL_SSIM_ANT", rcpmul_spec),
    )


DEN_SSIM, RCPMUL = _register_dve_ops()

F32 = mybir.dt.float32
F32R = mybir.dt.float32r
BF16 = mybir.dt.bfloat16
FP8 = mybir.dt.float8e4
ALU = mybir.AluOpType
ACT = mybir.ActivationFunctionType
DR = mybir.MatmulPerfMode.DoubleRow
np_fp8 = mybir.dt.np(FP8)

C1 = 0.2
WR, WG, WB = 0.299, 0.587, 0.114
C1T = C1 / (WG * WG)  # C1 for the (1/wg)-scaled gray values
RCP_C0 = -0.23549792
RCP_C1 = 2.0017324
K_LOSS = np.array([9, 8, 7, 6, 5, 4, 3, 2, 1], dtype=np.float64)  # K_LOSS[d]
N_CORES = 8
H = W = 1024

# acc columns: 16 for level-8 half-chunks, 4 for level-7 slabs, 1 for level-6
ACC_COLS = 21

LAST_RESULTS = None  # BassKernelResults of the most recent run (for profiling)

_CACHED_NC = None


def _ensure_ntff_hook():
    """Register the axon NTFF profile hook if the image's antenv lacks it."""
    try:
        from antenv.axon_hooks import get_axon_ntff_profile_hook

        return get_axon_ntff_profile_hook() is not None
    except ImportError:
        pass
    try:
        import types

        import antenv
        from trn_agent_boot.trn_boot import _ntff_profile_via_ctypes

        mod = types.ModuleType("antenv.axon_hooks")
        _h = {}
        mod.set_axon_ntff_profile_hook = lambda h: _h.__setitem__("h", h)
        mod.get_axon_ntff_profile_hook = lambda: _h.get("h")
        sys.modules["antenv.axon_hooks"] = mod
        antenv.axon_hooks = mod
        hook = _ntff_profile_via_ctypes("/opt/axon/libaxon_pjrt.so")
        mod.set_axon_ntff_profile_hook(hook)
        from concourse import bass_utils as _bu

        _bu.upload_artifacts = lambda tmpdir: tmpdir
        return hook is not None
    except Exception as e:  # pragma: no cover - profiling-only path
        print(f"ntff hook setup failed: {type(e).__name__}: {e}")
        return False


def _weight_matrices():
    """wdr [5,128,256] fp8: DoubleRow k-stacked [128, k=2, 128] weights
    (+I,+I), (-I,-I), (+I,0), (0,+I), (+I,-I).
    wpr [4,128,128] f32(r): Pa, Pb row-pair pooling, +I, -I."""
    eye = np.eye(128, dtype=np.float32)
    zero = np.zeros((128, 128), dtype=np.float32)
    def k2(a, b):
        return np.stack([a, b], axis=1).reshape(128, 256)
    wdr = np.stack(
        [k2(eye, eye), k2(-eye, -eye), k2(eye, zero), k2(zero, eye), k2(eye, -eye)]
    ).astype(np_fp8)
    wpr = np.zeros((4, 128, 128), dtype=np.float32)
    for j in range(64):
        wpr[0, 2 * j, j] = 1.0       # Pa: even chunk row pairs -> part 0..63
        wpr[0, 2 * j + 1, j] = 1.0
        wpr[1, 2 * j, 64 + j] = 1.0  # Pb: odd chunk row pairs -> part 64..127
        wpr[1, 2 * j + 1, 64 + j] = 1.0
    wpr[2] = eye
    wpr[3] = -eye
    return wdr, wpr


def _build_nc():
    nc = bacc.Bacc("TRN2", target_bir_lowering=False, debug=False)

    rgb_d = nc.declare_dram_parameter("rgbxy", [8, 128, 6144], FP8, isOutput=False)
    wdr_d = nc.declare_dram_parameter("wdr", [5, 128, 256], FP8, isOutput=False)
    wpr_d = nc.declare_dram_parameter("wpr", [4, 128, 128], F32R, isOutput=False)
    acc_d = nc.declare_dram_parameter("acc", [128, ACC_COLS], F32, isOutput=True)
    s5t5_d = nc.declare_dram_parameter("s5t5", [128, 256], F32, isOutput=True)

    with tile.TileContext(nc) as tc:
        with (
            tc.tile_pool(name="singles", bufs=1) as singles,
            tc.tile_pool(name="rgb", bufs=4) as rgb_pool,
            tc.tile_pool(name="gray", bufs=4) as gray_pool,
            tc.tile_pool(name="sd", bufs=2) as sd_pool,
            tc.tile_pool(name="pg", bufs=2, space="PSUM") as pg_pool,
            tc.tile_pool(name="pv", bufs=2, space="PSUM") as pv_pool,
            tc.tile_pool(name="pp", bufs=2, space="PSUM") as pp_pool,
        ):
            # --- weights (scalar queue: own HWDGE, parallel with inputs) ---
            wdr_t = [
                singles.tile([128, 256], FP8, tag=f"wdr{i}", name=f"wdr{i}")
                for i in range(5)
            ]
            for i in range(5):
                nc.scalar.dma_start(wdr_t[i][:], wdr_d[i])
            w_pp, w_nn, w_p0, w_0p, w_pn = [
                t[:].rearrange("p (k m) -> p k m", k=2) for t in wdr_t
            ]
            wpr_t = [
                singles.tile([128, 128], F32R, tag=f"wpr{i}", name=f"wpr{i}")
                for i in range(4)
            ]
            for i in range(4):
                nc.scalar.dma_start(wpr_t[i][:], wpr_d[i])
            pa, pb, ip, im = [t[:] for t in wpr_t]

            # --- inputs: one fp8 DMA per 128-row chunk ---
            rgb = [
                rgb_pool.tile([128, 6144], FP8, tag="rgb", name=f"rgb{j}")
                for j in range(8)
            ]
            for j in range(8):
                nc.sync.dma_start(rgb[j][:], rgb_d[j])

            acc = singles.tile([128, ACC_COLS], F32)
            s5t5 = singles.tile([128, 256], F32)
            s7all = singles.tile([128, 2048], F32, tag="s7all")
            t7all = singles.tile([128, 2048], F32, tag="t7all")
            s6all = singles.tile([128, 512], F32, tag="s6all")
            t6all = singles.tile([128, 512], F32, tag="t6all")
            dead = singles.tile([128, 2048], F32, tag="dead")

            def gray_chunk(j, col0):
                """Chunk j: s,t gray -> g8 [128, 2048] f32 (x|y) via DR
                matmuls + scalar evac; diff -> 2 PSUM slabs; then the two
                vector passes (den + rcpmul with psum diff) per 512-slab.
                acc cols col0, col0+1."""
                v = rgb[j][:].rearrange("p (xy c n) -> p xy c n", xy=2, c=3)
                g = gray_pool.tile([128, 2048], F32, tag="g8")
                pss = pg_pool.tile([128, 1024], F32, tag="pg", name="pss")
                pst = pg_pool.tile([128, 1024], F32, tag="pg", name="pst")
                slabs = (slice(0, 512), slice(512, 1024))
                # s,t gray: weight-major, each weight loaded once
                for sl in slabs:
                    nc.tensor.matmul(
                        pss[:, sl], w_pp, v[:, 0, 0:2, sl],
                        start=True, stop=False, perf_mode=DR,
                    )
                for sl in slabs:
                    nc.tensor.matmul(
                        pst[:, sl], w_pp, v[:, 1, 0:2, sl],
                        start=True, stop=False, perf_mode=DR,
                    )
                for sl in slabs:
                    nc.tensor.matmul(
                        pss[:, sl], w_p0, v[:, :, 2, sl],
                        start=False, stop=True, perf_mode=DR,
                    )
                for sl in slabs:
                    nc.tensor.matmul(
                        pst[:, sl], w_0p, v[:, :, 2, sl],
                        start=False, stop=True, perf_mode=DR,
                    )
                nc.scalar.activation(g[:, 0:1024].bitcast(F32R), pss[:], ACT.Copy)
                nc.scalar.activation(g[:, 1024:2048].bitcast(F32R), pst[:], ACT.Copy)
                # diff = gray_x - gray_y, one PSUM slab at a time
                den = sd_pool.tile([128, 1024], F32, tag="den8")
                nc.vector._custom_dve(
                    DEN_SSIM, out=den[:], in0=g[:, 0:1024], in1=g[:, 1024:2048],
                    s0=C1T,
                )
                for si, sl in enumerate(slabs):
                    pvv = pv_pool.tile([128, 512], F32, tag="pv", name="pvv")
                    nc.tensor.matmul(
                        pvv[:], w_pp, v[:, 0, 0:2, sl],
                        start=True, stop=False, perf_mode=DR,
                    )
                    nc.tensor.matmul(
                        pvv[:], w_nn, v[:, 1, 0:2, sl],
                        start=False, stop=False, perf_mode=DR,
                    )
                    nc.tensor.matmul(
                        pvv[:], w_pn, v[:, :, 2, sl],
                        start=False, stop=True, perf_mode=DR,
                    )
                    nc.vector._custom_dve(
                        RCPMUL,
                        out=dead[:, 0:512],
                        in0=den[:, sl],
                        in1=pvv[:],
                        s0=RCP_C0,
                        s1=RCP_C1,
                        accum_out=acc[:, col0 + si : col0 + si + 1],
                    )
                return g

            def pool_pair(even_ap, odd_ap, fd, out_ap, round_f32r=True):
                """2x2 sum-pool two stacked [128, fd] f32 chunks ->
                out_ap [128, fd//2].  Row pairs via Pa/Pb f32r matmuls,
                column pairs via stride-2 rhs views accumulating into one
                PSUM region; scalar engine evacuates."""
                ps = pp_pool.tile([128, 512], F32, tag="pp", name="ps")
                half = fd // 2
                seq = (
                    (pa, even_ap[:, 0:fd:2]),
                    (pa, even_ap[:, 1:fd:2]),
                    (pb, odd_ap[:, 0:fd:2]),
                    (pb, odd_ap[:, 1:fd:2]),
                )
                for i, (wm, src) in enumerate(seq):
                    nc.tensor.matmul(
                        ps[:, 0:half], wm, src.bitcast(F32R),
                        start=(i == 0), stop=(i == 3),
                    )
                if round_f32r:
                    out_ap = out_ap.bitcast(F32R)
                nc.scalar.activation(out_ap, ps[:, 0:half], ACT.Copy)

            def ssim_sbuf(s_ap, t_ap, fd, col0, tag):
                """SSIM level with s,t in SBUF: diff via +-I f32r matmuls
                into PSUM 512-slabs, den on vector, rcpmul per slab."""
                den = sd_pool.tile([128, fd], F32, tag=f"den{tag}")
                nc.vector._custom_dve(
                    DEN_SSIM, out=den[:], in0=s_ap, in1=t_ap, s0=C1T
                )
                n_slab = fd // 512
                for k in range(max(n_slab, 1)):
                    w = min(512, fd)
                    sl = slice(512 * k, 512 * k + w)
                    pvv = pv_pool.tile([128, 512], F32, tag="pv", name="pvl")
                    nc.tensor.matmul(
                        pvv[:, 0:w], ip, s_ap[:, sl].bitcast(F32R),
                        start=True, stop=False,
                    )
                    nc.tensor.matmul(
                        pvv[:, 0:w], im, t_ap[:, sl].bitcast(F32R),
                        start=False, stop=True,
                    )
                    nc.vector._custom_dve(
                        RCPMUL,
                        out=dead[:, 0:w],
                        in0=den[:, sl],
                        in1=pvv[:, 0:w],
                        s0=RCP_C0,
                        s1=RCP_C1,
                        accum_out=acc[:, col0 + k : col0 + k + 1],
                    )

            # ---- level 8: gray+ssim per chunk, pool pairs to level 7 ----
            g8 = [None] * 8
            for j in range(8):
                g8[j] = gray_chunk(j, 2 * j)
                if j % 2 == 1:
                    k = j // 2
                    ge, go = g8[j - 1], g8[j]
                    pool_pair(
                        ge[:, 0:1024], go[:, 0:1024], 1024,
                        s7all[:, 512 * k : 512 * (k + 1)],
                    )
                    pool_pair(
                        ge[:, 1024:2048], go[:, 1024:2048], 1024,
                        t7all[:, 512 * k : 512 * (k + 1)],
                    )

            # ---- level 7 ([128, 2048]) + pool to level 6 ----
            ssim_sbuf(s7all[:], t7all[:], 2048, 16, "7")
            for k in range(2):
                pool_pair(
                    s7all[:, 1024 * k : 1024 * k + 512],
                    s7all[:, 1024 * k + 512 : 1024 * (k + 1)],
                    512,
                    s6all[:, 256 * k : 256 * (k + 1)],
                )
                pool_pair(
                    t7all[:, 1024 * k : 1024 * k + 512],
                    t7all[:, 1024 * k + 512 : 1024 * (k + 1)],
                    512,
                    t6all[:, 256 * k : 256 * (k + 1)],
                )

            # ---- level 6 ([128, 512]) + pool to level 5 ----
            ssim_sbuf(s6all[:], t6all[:], 512, 20, "6")
            pool_pair(
                s6all[:, 0:256], s6all[:, 256:512], 256, s5t5[:, 0:128],
                round_f32r=False,
            )
            pool_pair(
                t6all[:, 0:256], t6all[:, 256:512], 256, s5t5[:, 128:256],
                round_f32r=False,
            )

            nc.sync.dma_start(acc_d[:], acc[:])
            nc.sync.dma_start(s5t5_d[:], s5t5[:])

    nc.compile()
    return nc


def _get_nc():
    global _CACHED_NC
    if _CACHED_NC is None:
        _CACHED_NC = _build_nc()
    return _CACHED_NC


def _host_tail(per_core):
    """Combine per-core results into the scalar loss (float64 host math)."""
    total = 0.0
    # device levels: 8 (acc cols 0..15), 7 (16..19), 6 (20)
    for d, cols in ((8, slice(0, 16)), (7, slice(16, 20)), (6, slice(20, 21))):
        s = sum(float(r["acc"][:, cols].astype(np.float64).sum()) for r in per_core)
        cnt = N_CORES * 16 * 4**d
        total += K_LOSS[d] * (s / cnt)
    # host levels: 5..0 on the shipped pooled images ((1/wg)-scaled values)
    s = np.stack([r["s5t5"][:, 0:128] for r in per_core]).astype(np.float64)
    t = np.stack([r["s5t5"][:, 128:256] for r in per_core]).astype(np.float64)
    for d in range(5, -1, -1):
        ratio = (s - t) ** 2 / (s * s + t * t + C1T)
        cnt = N_CORES * 16 * 4**d
        total += K_LOSS[d] * (ratio.sum() / cnt)
        if d > 0:
            b, n, _ = s.shape
            s = s.reshape(b, n // 2, 2, n // 2, 2).sum(axis=(2, 4))
            t = t.reshape(b, n // 2, 2, n // 2, 2).sum(axis=(2, 4))
    return np.float32(total)


def _pack_inputs(input, target):
    """[8,3,1024,1024] f32 x2 -> per-core [8,128,6144] fp8e4m3.
    Partition line layout [x|y][R',B',G][1024]; R,B pre-scaled by their
    grayscale weights so device mix weights are exactly +-1."""
    scale = np.array([WR / WG, WB / WG, 1.0], dtype=np.float32)[:, None, None]
    # reorder channels to (R, B, G) then scale
    xin = input[:, (0, 2, 1)] * scale
    yin = target[:, (0, 2, 1)] * scale
    out = np.empty((N_CORES, 8, 128, 2, 3, 1024), dtype=np_fp8)
    out[:, :, :, 0, :, :] = xin.reshape(8, 3, 8, 128, 1024).transpose(0, 2, 3, 1, 4)
    out[:, :, :, 1, :, :] = yin.reshape(8, 3, 8, 128, 1024).transpose(0, 2, 3, 1, 4)
    return out.reshape(N_CORES, 8, 128, 6144)


def kernel(input, target):
    global LAST_RESULTS
    input = np.ascontiguousarray(np.asarray(input, dtype=np.float32))
    target = np.ascontiguousarray(np.asarray(target, dtype=np.float32))
    assert input.shape == (N_CORES, 3, H, W), input.shape

    nc = _get_nc()
    rgbxy = _pack_inputs(input, target)
    wdr, wpr = _weight_matrices()
    in_maps = [
        {"rgbxy": rgbxy[i], "wdr": wdr, "wpr": wpr} for i in range(N_CORES)
    ]
    trace = bool(int(os.environ.get("BASS_SSIM_TRACE", "0")))
    if trace:
        trace = _ensure_ntff_hook()
    res = run_bass_kernel_spmd(nc, in_maps, list(range(N_CORES)), trace=trace)
    LAST_RESULTS = res
    return _host_tail(res.results)


# revision 28
# speedup vs baseline: 1.0820x; 1.0820x over previous
"""Trainium2 Bass kernel for nn_DividedSsimLoss.

Reference: for 8 RGB 1024x1024 image pairs, grayscale, tile 256x256,
9-level 2x2 sum-pool pyramid, loss = sum_d K[d] * (1 - mean ssim_d),
ssim = (2st + C1) / (s^2 + t^2 + C1), i.e. 1-ssim = (s-t)^2/(s^2+t^2+C1).

v4 design (per core = one image pair, pure data parallelism):
  * Host ships the 6 channel planes as fp8e4m3, with R and B pre-scaled
    by their grayscale weights (wr/wg, wb/wg) so that every device-side
    channel-mix weight is exactly +-1.0 in fp8.  Layout per partition
    line: [x|y][R',B',G][1024] -> one [128, 6144] fp8 DMA per 128-row
    chunk (6 KiB contiguous DRAM per partition line).  6 MiB per core.
  * Grayscale (scaled 1/wg) runs on the tensor engine with fp8
    DoubleRow matmuls: one DR matmul contracts the stacked [R'; B']
    2-k-tile pair at 0.5 cyc/row, a second DR matmul adds G via a
    [I; 0] / [0; I] weight pair.  PSUM accumulates in f32; the scalar
    engine evacuates to SBUF as f32r-rounded f32.
  * diff = gray_x - gray_y is ALSO computed on the tensor engine
    ([I;I] / [-I;-I] / [I;-I] DR matmuls into separate PSUM slabs),
    exactly consistent with s - t.
  * Per level the vector engine runs only 2 custom DVE passes:
    DEN = s^2+t^2+C1 (from SBUF) and the fused
    RCPMUL: accum += diff^2 * recip_approx(den), with diff read
    directly from PSUM (one PSUM operand is allowed).
  * 2x2 pooling: row pairs via Pa/Pb f32r matmuls with stride-2 rhs
    views (column pairs fold into the same PSUM accumulation), scalar
    engine evacuates.  Levels 7/6 pyramid diffs via +-I f32r matmuls.
  * Device covers levels 8,7,6 + pooled level-5 images; host does
    levels 5..0 in f64.
"""

import os
import sys

import numpy as np

for _p in ("/opt/trn_rl_repo",):
    if _p not in sys.path:
        sys.path.insert(0, _p)

import concourse.bacc as bacc
import concourse.bass as bass
import concourse.mybir as mybir
import concourse.tile as tile
from concourse.bass_utils import run_bass_kernel_spmd

from ml_dtypes import bfloat16 as np_bf16


def _register_dve_ops():
    """Register kernel-specific custom DVE ops (idempotent).

    DEN_SSIM:  out = in0^2 + in1^2 + s0
    RCPMUL:    out = in1^2 * y1(in0),  accum = sum(out)
               y1 = one-NR reciprocal approx of in0 (bitwise-not seed)
    """
    import concourse.dve_ops as dve_ops
    from concourse.dve_ops import DveOp
    from concourse.dve_spec import (
        C0,
        C1,
        AluOp,
        Bin,
        Spec,
        Src0,
        Src1,
        _has_src1,
        lower,
        sq,
    )
    from concourse.dve_uop import DveOpSpec
    from operator import add as _add

    def _sha_for(name, spec):
        shas = {}
        for ver in ("v3",):
            row = dve_ops._SUB_OPCODE_FOR_NAME[name]
            s = DveOpSpec(
                name=name, opcode=row, uops=lower(spec, ver=ver),
                rd1_en=_has_src1(spec),
            )
            shas[ver] = s.sha(ver)
        return shas

    def _register(name, spec):
        if name in dve_ops._SUB_OPCODE_FOR_NAME:
            return next(op for op in dve_ops.OPS if op.name == name)
        row = dve_ops._CUSTOM_DVE_ROW_BASE + len(dve_ops.OPS)
        assert row < 0x20, "custom-DVE row field overflow"
        dve_ops._SUB_OPCODE_FOR_NAME[name] = row
        op = DveOp(name, spec, subdim=False, uops_sha=_sha_for(name, spec))
        dve_ops.OPS.append(op)
        dve_ops.CUSTOM_DVE_SPECS[name] = spec
        return op

    den_spec = Spec(
        body=sq(Src0) + sq(Src1) + C0,
        reference=lambda in0, in1, s0, s1, imm2: (
            in0.astype(np.float32) ** 2 + in1.astype(np.float32) ** 2 + s0
        ),
    )

    # reciprocal seed: x * bitcast(~x) lands in [-4.5, -4]; one Chebyshev
    # scale + one NR pass (same constants as RECIPROCAL_APPROX_FAST).
    _nx = Bin(AluOp.BITWISE_NOT, Src0, Src0)
    _y0 = _nx * C0
    _y1 = _y0 * (C1 - Src0 * _y0)

    def _ref_rcpmul(in0, in1, c0, c1, c2):
        not_x = (~in0.astype(np.float32).view(np.int32)).view(np.float32)
        y0 = not_x * c0
        y1 = y0 * (c1 - in0.astype(np.float32) * y0)
        return in1.astype(np.float32) ** 2 * y1

    rcpmul_spec = Spec(
        body=sq(Src1) * _y1,
        accum=_add,
        reference=dve_ops._ref_body_sum(_ref_rcpmul),
    )

    return (
        _register("DEN_SSIM_ANT", den_spec),
        _register("RCPMUL_SSIM_ANT", rcpmul_spec),
    )


DEN_SSIM, RCPMUL = _register_dve_ops()

F32 = mybir.dt.float32
F32R = mybir.dt.float32r
BF16 = mybir.dt.bfloat16
FP8 = mybir.dt.float8e4
ALU = mybir.AluOpType
ACT = mybir.ActivationFunctionType
DR = mybir.MatmulPerfMode.DoubleRow
np_fp8 = mybir.dt.np(FP8)

C1 = 0.2
WR, WG, WB = 0.299, 0.587, 0.114
C1T = C1 / (WG * WG)  # C1 for the (1/wg)-scaled gray values
RCP_C0 = -0.23549792
RCP_C1 = 2.0017324
K_LOSS = np.array([9, 8, 7, 6, 5, 4, 3, 2, 1], dtype=np.float64)  # K_LOSS[d]
N_CORES = 8
H = W = 1024

# acc columns: 16 for level-8 half-chunks, 4 for level-7 slabs, 1 for level-6
ACC_COLS = 21

LAST_RESULTS = None  # BassKernelResults of the most recent run (for profiling)

_CACHED_NC = None


def _ensure_ntff_hook():
    """Register the axon NTFF profile hook if the image's antenv lacks it."""
    try:
        from antenv.axon_hooks import get_axon_ntff_profile_hook

        return get_axon_ntff_profile_hook() is not None
    except ImportError:
        pass
    try:
        import types

        import antenv
        from trn_agent_boot.trn_boot import _ntff_profile_via_ctypes

        mod = types.ModuleType("antenv.axon_hooks")
        _h = {}
        mod.set_axon_ntff_profile_hook = lambda h: _h.__setitem__("h", h)
        mod.get_axon_ntff_profile_hook = lambda: _h.get("h")
        sys.modules["antenv.axon_hooks"] = mod
        antenv.axon_hooks = mod
        hook = _ntff_profile_via_ctypes("/opt/axon/libaxon_pjrt.so")
        mod.set_axon_ntff_profile_hook(hook)
        from concourse import bass_utils as _bu

        _bu.upload_artifacts = lambda tmpdir: tmpdir
        return hook is not None
    except Exception as e:  # pragma: no cover - profiling-only path
        print(f"ntff hook setup failed: {type(e).__name__}: {e}")
        return False


def _weight_matrices():
    """wdr [5,128,256] fp8: DoubleRow k-stacked [128, k=2, 128] weights
    (+I,+I), (-I,-I), (+I,0), (0,+I), (+I,-I).
    wpr [4,128,128] f32(r): Pa, Pb row-pair pooling, +I, -I."""
    eye = np.eye(128, dtype=np.float32)
    zero = np.zeros((128, 128), dtype=np.float32)
    def k2(a, b):
        return np.stack([a, b], axis=1).reshape(128, 256)
    wdr = np.stack(
        [k2(eye, eye), k2(-eye, -eye), k2(eye, zero), k2(zero, eye), k2(eye, -eye)]
    ).astype(np_fp8)
    wpr = np.zeros((4, 128, 128), dtype=np.float32)
    for j in range(64):
        wpr[0, 2 * j, j] = 1.0       # Pa: even chunk row pairs -> part 0..63
        wpr[0, 2 * j + 1, j] = 1.0
        wpr[1, 2 * j, 64 + j] = 1.0  # Pb: odd chunk row pairs -> part 64..127
        wpr[1, 2 * j + 1, 64 + j] = 1.0
    wpr[2] = eye
    wpr[3] = -eye
    return wdr, wpr


def _build_nc():
    nc = bacc.Bacc("TRN2", target_bir_lowering=False, debug=False)

    rgb_d = nc.declare_dram_parameter("rgbxy", [8, 128, 6144], FP8, isOutput=False)
    wdr_d = nc.declare_dram_parameter("wdr", [5, 128, 256], FP8, isOutput=False)
    wpr_d = nc.declare_dram_parameter("wpr", [4, 128, 128], F32R, isOutput=False)
    acc_d = nc.declare_dram_parameter("acc", [128, ACC_COLS], F32, isOutput=True)
    s5t5_d = nc.declare_dram_parameter("s5t5", [128, 256], F32, isOutput=True)

    with tile.TileContext(nc) as tc:
        with (
            tc.tile_pool(name="singles", bufs=1) as singles,
            tc.tile_pool(name="rgb", bufs=4) as rgb_pool,
            tc.tile_pool(name="gray", bufs=4) as gray_pool,
            tc.tile_pool(name="sd", bufs=2) as sd_pool,
            tc.tile_pool(name="pg", bufs=2, space="PSUM") as pg_pool,
            tc.tile_pool(name="pv", bufs=2, space="PSUM") as pv_pool,
            tc.tile_pool(name="pp", bufs=2, space="PSUM") as pp_pool,
        ):
            # --- weights (scalar queue: own HWDGE, parallel with inputs) ---
            wdr_t = [
                singles.tile([128, 256], FP8, tag=f"wdr{i}", name=f"wdr{i}")
                for i in range(5)
            ]
            for i in range(5):
                nc.scalar.dma_start(wdr_t[i][:], wdr_d[i])
            w_pp, w_nn, w_p0, w_0p, w_pn = [
                t[:].rearrange("p (k m) -> p k m", k=2) for t in wdr_t
            ]
            wpr_t = [
                singles.tile([128, 128], F32R, tag=f"wpr{i}", name=f"wpr{i}")
                for i in range(4)
            ]
            for i in range(4):
                nc.scalar.dma_start(wpr_t[i][:], wpr_d[i])
            pa, pb, ip, im = [t[:] for t in wpr_t]

            # --- inputs: one fp8 DMA per 128-row chunk ---
            rgb = [
                rgb_pool.tile([128, 6144], FP8, tag="rgb", name=f"rgb{j}")
                for j in range(8)
            ]
            for j in range(8):
                nc.sync.dma_start(rgb[j][:], rgb_d[j])

            acc = singles.tile([128, ACC_COLS], F32)
            s5t5 = singles.tile([128, 256], F32)
            s7all = singles.tile([128, 2048], F32, tag="s7all")
            t7all = singles.tile([128, 2048], F32, tag="t7all")
            s6all = singles.tile([128, 512], F32, tag="s6all")
            t6all = singles.tile([128, 512], F32, tag="t6all")
            dead = singles.tile([128, 2048], F32, tag="dead")

            def gray_chunk(j, col0):
                """Chunk j: s,t gray -> g8 [128, 2048] f32 (x|y) via DR
                matmuls + scalar evac; diff -> 2 PSUM slabs; then the two
                vector passes (den + rcpmul with psum diff) per 512-slab.
                acc cols col0, col0+1."""
                v = rgb[j][:].rearrange("p (xy c n) -> p xy c n", xy=2, c=3)
                g = gray_pool.tile([128, 2048], F32, tag="g8")
                pss = pg_pool.tile([128, 1024], F32, tag="pg", name="pss")
                pst = pg_pool.tile([128, 1024], F32, tag="pg", name="pst")
                slabs = (slice(0, 512), slice(512, 1024))
                # s,t gray: weight-major, each weight loaded once
                for sl in slabs:
                    nc.tensor.matmul(
                        pss[:, sl], w_pp, v[:, 0, 0:2, sl],
                        start=True, stop=False, perf_mode=DR,
                    )
                for sl in slabs:
                    nc.tensor.matmul(
                        pst[:, sl], w_pp, v[:, 1, 0:2, sl],
                        start=True, stop=False, perf_mode=DR,
                    )
                for sl in slabs:
                    nc.tensor.matmul(
                        pss[:, sl], w_p0, v[:, :, 2, sl],
                        start=False, stop=True, perf_mode=DR,
                    )
                for sl in slabs:
                    nc.tensor.matmul(
                        pst[:, sl], w_0p, v[:, :, 2, sl],
                        start=False, stop=True, perf_mode=DR,
                    )
                nc.scalar.activation(g[:, 0:1024].bitcast(F32R), pss[:], ACT.Copy)
                nc.scalar.activation(g[:, 1024:2048].bitcast(F32R), pst[:], ACT.Copy)
                # diff = gray_x - gray_y, one PSUM slab at a time
                den = sd_pool.tile([128, 1024], F32, tag="den8")
                nc.vector._custom_dve(
                    DEN_SSIM, out=den[:], in0=g[:, 0:1024], in1=g[:, 1024:2048],
                    s0=C1T,
                )
                for si, sl in enumerate(slabs):
                    pvv = pv_pool.tile([128, 512], F32, tag="pv", name="pvv")
                    nc.tensor.matmul(
                        pvv[:], w_pp, v[:, 0, 0:2, sl],
                        start=True, stop=False, perf_mode=DR,
                    )
                    nc.tensor.matmul(
                        pvv[:], w_nn, v[:, 1, 0:2, sl],
                        start=False, stop=False, perf_mode=DR,
                    )
                    nc.tensor.matmul(
                        pvv[:], w_pn, v[:, :, 2, sl],
                        start=False, stop=True, perf_mode=DR,
                    )
                    nc.vector._custom_dve(
                        RCPMUL,
                        out=dead[:, 0:512],
                        in0=den[:, sl],
                        in1=pvv[:],
                        s0=RCP_C0,
                        s1=RCP_C1,
                        accum_out=acc[:, col0 + si : col0 + si + 1],
                    )
                return g

            def pool_pair(even_ap, odd_ap, fd, out_ap, round_f32r=True):
                """2x2 sum-pool two stacked [128, fd] f32 chunks ->
                out_ap [128, fd//2].  Row pairs via Pa/Pb f32r matmuls,
                column pairs via stride-2 rhs views accumulating into one
                PSUM region; scalar engine evacuates."""
                ps = pp_pool.tile([128, 512], F32, tag="pp", name="ps")
                half = fd // 2
                seq = (
                    (pa, even_ap[:, 0:fd:2]),
                    (pa, even_ap[:, 1:fd:2]),
                    (pb, odd_ap[:, 0:fd:2]),
                    (pb, odd_ap[:, 1:fd:2]),
                )
                for i, (wm, src) in enumerate(seq):
                    nc.tensor.matmul(
                        ps[:, 0:half], wm, src.bitcast(F32R),
                        start=(i == 0), stop=(i == 3),
                    )
                if round_f32r:
                    out_ap = out_ap.bitcast(F32R)
                nc.scalar.activation(out_ap, ps[:, 0:half], ACT.Copy)

            def ssim_sbuf(s_ap, t_ap, fd, col0, tag):
                """SSIM level with s,t in SBUF: diff via +-I f32r matmuls
                into PSUM 512-slabs, den on vector, rcpmul per slab."""
                den = sd_pool.tile([128, fd], F32, tag=f"den{tag}")
                nc.vector._custom_dve(
                    DEN_SSIM, out=den[:], in0=s_ap, in1=t_ap, s0=C1T
                )
                n_slab = fd // 512
                for k in range(max(n_slab, 1)):
                    w = min(512, fd)
                    sl = slice(512 * k, 512 * k + w)
                    pvv = pv_pool.tile([128, 512], F32, tag="pv", name="pvl")
                    nc.tensor.matmul(
                        pvv[:, 0:w], ip, s_ap[:, sl].bitcast(F32R),
                        start=True, stop=False,
                    )
                    nc.tensor.matmul(
                        pvv[:, 0:w], im, t_ap[:, sl].bitcast(F32R),
                        start=False, stop=True,
                    )
                    nc.vector._custom_dve(
                        RCPMUL,
                        out=dead[:, 0:w],
                        in0=den[:, sl],
                        in1=pvv[:, 0:w],
                        s0=RCP_C0,
                        s1=RCP_C1,
                        accum_out=acc[:, col0 + k : col0 + k + 1],
                    )

            def colpool(src_ap, fd, tag):
                """GpSimd: add adjacent column pairs, [128,fd] -> [128,fd/2]."""
                c = sd_pool.tile([128, fd // 2], F32R, tag=f"cp{tag}", name="cp")
                nc.gpsimd.tensor_tensor(
                    c[:], src_ap[:, 0:fd:2], src_ap[:, 1:fd:2], ALU.add
                )
                return c

            def pool_pair_cf(even_ap, odd_ap, fd, out_ap, tag):
                """2x2 sum-pool, colpool-first variant: column pairs on
                gpsimd, row pairs via 2 contiguous-rhs f32r matmuls."""
                half = fd // 2
                ce = colpool(even_ap, fd, tag + "e")
                co = colpool(odd_ap, fd, tag + "o")
                ps = pp_pool.tile([128, 512], F32, tag="pp", name="psc")
                nc.tensor.matmul(ps[:, 0:half], pa, ce[:], start=True, stop=False)
                nc.tensor.matmul(ps[:, 0:half], pb, co[:], start=False, stop=True)
                nc.scalar.activation(out_ap.bitcast(F32R), ps[:, 0:half], ACT.Copy)

            # ---- level 8: gray+ssim per chunk, pool pairs to level 7 ----
            g8 = [None] * 8
            for j in range(8):
                g8[j] = gray_chunk(j, 2 * j)
                if j % 2 == 1:
                    k = j // 2
                    ge, go = g8[j - 1], g8[j]
                    pool_pair_cf(
                        ge[:, 0:1024], go[:, 0:1024], 1024,
                        s7all[:, 512 * k : 512 * (k + 1)], f"s{k}",
                    )
                    pool_pair_cf(
                        ge[:, 1024:2048], go[:, 1024:2048], 1024,
                        t7all[:, 512 * k : 512 * (k + 1)], f"t{k}",
                    )

            # ---- level 7 ([128, 2048]) + pool to level 6 ----
            ssim_sbuf(s7all[:], t7all[:], 2048, 16, "7")
            for k in range(2):
                pool_pair(
                    s7all[:, 1024 * k : 1024 * k + 512],
                    s7all[:, 1024 * k + 512 : 1024 * (k + 1)],
                    512,
                    s6all[:, 256 * k : 256 * (k + 1)],
                )
                pool_pair(
                    t7all[:, 1024 * k : 1024 * k + 512],
                    t7all[:, 1024 * k + 512 : 1024 * (k + 1)],
                    512,
                    t6all[:, 256 * k : 256 * (k + 1)],
                )

            # ---- level 6 ([128, 512]) + pool to level 5 ----
            ssim_sbuf(s6all[:], t6all[:], 512, 20, "6")
            pool_pair(
                s6all[:, 0:256], s6all[:, 256:512], 256, s5t5[:, 0:128],
                round_f32r=False,
            )
            pool_pair(
                t6all[:, 0:256], t6all[:, 256:512], 256, s5t5[:, 128:256],
                round_f32r=False,
            )

            nc.sync.dma_start(acc_d[:], acc[:])
            nc.sync.dma_start(s5t5_d[:], s5t5[:])

    nc.compile()
    return nc


def _get_nc():
    global _CACHED_NC
    if _CACHED_NC is None:
        _CACHED_NC = _build_nc()
    return _CACHED_NC


def _host_tail(per_core):
    """Combine per-core results into the scalar loss (float64 host math)."""
    total = 0.0
    # device levels: 8 (acc cols 0..15), 7 (16..19), 6 (20)
    for d, cols in ((8, slice(0, 16)), (7, slice(16, 20)), (6, slice(20, 21))):
        s = sum(float(r["acc"][:, cols].astype(np.float64).sum()) for r in per_core)
        cnt = N_CORES * 16 * 4**d
        total += K_LOSS[d] * (s / cnt)
    # host levels: 5..0 on the shipped pooled images ((1/wg)-scaled values)
    s = np.stack([r["s5t5"][:, 0:128] for r in per_core]).astype(np.float64)
    t = np.stack([r["s5t5"][:, 128:256] for r in per_core]).astype(np.float64)
    for d in range(5, -1, -1):
        ratio = (s - t) ** 2 / (s * s + t * t + C1T)
        cnt = N_CORES * 16 * 4**d
        total += K_LOSS[d] * (ratio.sum() / cnt)
        if d > 0:
            b, n, _ = s.shape
            s = s.reshape(b, n // 2, 2, n // 2, 2).sum(axis=(2, 4))
            t = t.reshape(b, n // 2, 2, n // 2, 2).sum(axis=(2, 4))
    return np.float32(total)


def _pack_inputs(input, target):
    """[8,3,1024,1024] f32 x2 -> per-core [8,128,6144] fp8e4m3.
    Partition line layout [x|y][R',B',G][1024]; R,B pre-scaled by their
    grayscale weights so device mix weights are exactly +-1."""
    scale = np.array([WR / WG, WB / WG, 1.0], dtype=np.float32)[:, None, None]
    # reorder channels to (R, B, G) then scale
    xin = input[:, (0, 2, 1)] * scale
    yin = target[:, (0, 2, 1)] * scale
    out = np.empty((N_CORES, 8, 128, 2, 3, 1024), dtype=np_fp8)
    out[:, :, :, 0, :, :] = xin.reshape(8, 3, 8, 128, 1024).transpose(0, 2, 3, 1, 4)
    out[:, :, :, 1, :, :] = yin.reshape(8, 3, 8, 128, 1024).transpose(0, 2, 3, 1, 4)
    return out.reshape(N_CORES, 8, 128, 6144)


def kernel(input, target):
    global LAST_RESULTS
    input = np.ascontiguousarray(np.asarray(input, dtype=np.float32))
    target = np.ascontiguousarray(np.asarray(target, dtype=np.float32))
    assert input.shape == (N_CORES, 3, H, W), input.shape

    nc = _get_nc()
    rgbxy = _pack_inputs(input, target)
    wdr, wpr = _weight_matrices()
    in_maps = [
        {"rgbxy": rgbxy[i], "wdr": wdr, "wpr": wpr} for i in range(N_CORES)
    ]
    trace = bool(int(os.environ.get("BASS_SSIM_TRACE", "0")))
    if trace:
        trace = _ensure_ntff_hook()
    res = run_bass_kernel_spmd(nc, in_maps, list(range(N_CORES)), trace=trace)
    LAST_RESULTS = res
    return _host_tail(res.results)


# revision 32
# speedup vs baseline: 1.1073x; 1.0234x over previous
"""Trainium2 Bass kernel for nn_DividedSsimLoss.

Reference: for 8 RGB 1024x1024 image pairs, grayscale, tile 256x256,
9-level 2x2 sum-pool pyramid, loss = sum_d K[d] * (1 - mean ssim_d),
ssim = (2st + C1) / (s^2 + t^2 + C1), i.e. 1-ssim = (s-t)^2/(s^2+t^2+C1).

v4 design (per core = one image pair, pure data parallelism):
  * Host ships the 6 channel planes as fp8e4m3, with R and B pre-scaled
    by their grayscale weights (wr/wg, wb/wg) so that every device-side
    channel-mix weight is exactly +-1.0 in fp8.  Layout per partition
    line: [x|y][R',B',G][1024] -> one [128, 6144] fp8 DMA per 128-row
    chunk (6 KiB contiguous DRAM per partition line).  6 MiB per core.
  * Grayscale (scaled 1/wg) runs on the tensor engine with fp8
    DoubleRow matmuls: one DR matmul contracts the stacked [R'; B']
    2-k-tile pair at 0.5 cyc/row, a second DR matmul adds G via a
    [I; 0] / [0; I] weight pair.  PSUM accumulates in f32; the scalar
    engine evacuates to SBUF as f32r-rounded f32.
  * diff = gray_x - gray_y is ALSO computed on the tensor engine
    ([I;I] / [-I;-I] / [I;-I] DR matmuls into separate PSUM slabs),
    exactly consistent with s - t.
  * Per level the vector engine runs only 2 custom DVE passes:
    DEN = s^2+t^2+C1 (from SBUF) and the fused
    RCPMUL: accum += diff^2 * recip_approx(den), with diff read
    directly from PSUM (one PSUM operand is allowed).
  * 2x2 pooling: row pairs via Pa/Pb f32r matmuls with stride-2 rhs
    views (column pairs fold into the same PSUM accumulation), scalar
    engine evacuates.  Levels 7/6 pyramid diffs via +-I f32r matmuls.
  * Device covers levels 8,7,6 + pooled level-5 images; host does
    levels 5..0 in f64.
"""

import os
import sys

import numpy as np

for _p in ("/opt/trn_rl_repo",):
    if _p not in sys.path:
        sys.path.insert(0, _p)

import concourse.bacc as bacc
import concourse.bass as bass
import concourse.mybir as mybir
import concourse.tile as tile
from concourse.bass_utils import run_bass_kernel_spmd

from ml_dtypes import bfloat16 as np_bf16


def _register_dve_ops():
    """Register kernel-specific custom DVE ops (idempotent).

    DEN_SSIM:  out = in0^2 + in1^2 + s0
    RCPMUL:    out = in1^2 * y1(in0),  accum = sum(out)
               y1 = one-NR reciprocal approx of in0 (bitwise-not seed)
    """
    import concourse.dve_ops as dve_ops
    from concourse.dve_ops import DveOp
    from concourse.dve_spec import (
        C0,
        C1,
        AluOp,
        Bin,
        Spec,
        Src0,
        Src1,
        _has_src1,
        lower,
        sq,
    )
    from concourse.dve_uop import DveOpSpec
    from operator import add as _add

    def _sha_for(name, spec):
        shas = {}
        for ver in ("v3",):
            row = dve_ops._SUB_OPCODE_FOR_NAME[name]
            s = DveOpSpec(
                name=name, opcode=row, uops=lower(spec, ver=ver),
                rd1_en=_has_src1(spec),
            )
            shas[ver] = s.sha(ver)
        return shas

    def _register(name, spec):
        if name in dve_ops._SUB_OPCODE_FOR_NAME:
            return next(op for op in dve_ops.OPS if op.name == name)
        row = dve_ops._CUSTOM_DVE_ROW_BASE + len(dve_ops.OPS)
        assert row < 0x20, "custom-DVE row field overflow"
        dve_ops._SUB_OPCODE_FOR_NAME[name] = row
        op = DveOp(name, spec, subdim=False, uops_sha=_sha_for(name, spec))
        dve_ops.OPS.append(op)
        dve_ops.CUSTOM_DVE_SPECS[name] = spec
        return op

    den_spec = Spec(
        body=sq(Src0) + sq(Src1) + C0,
        reference=lambda in0, in1, s0, s1, imm2: (
            in0.astype(np.float32) ** 2 + in1.astype(np.float32) ** 2 + s0
        ),
    )

    # reciprocal seed: x * bitcast(~x) lands in [-4.5, -4]; one Chebyshev
    # scale + one NR pass (same constants as RECIPROCAL_APPROX_FAST).
    _nx = Bin(AluOp.BITWISE_NOT, Src0, Src0)
    _y0 = _nx * C0
    _y1 = _y0 * (C1 - Src0 * _y0)

    def _ref_rcpmul(in0, in1, c0, c1, c2):
        not_x = (~in0.astype(np.float32).view(np.int32)).view(np.float32)
        y0 = not_x * c0
        y1 = y0 * (c1 - in0.astype(np.float32) * y0)
        return in1.astype(np.float32) ** 2 * y1

    rcpmul_spec = Spec(
        body=sq(Src1) * _y1,
        accum=_add,
        reference=dve_ops._ref_body_sum(_ref_rcpmul),
    )

    return (
        _register("DEN_SSIM_ANT", den_spec),
        _register("RCPMUL_SSIM_ANT", rcpmul_spec),
    )


DEN_SSIM, RCPMUL = _register_dve_ops()

F32 = mybir.dt.float32
F32R = mybir.dt.float32r
BF16 = mybir.dt.bfloat16
FP8 = mybir.dt.float8e4
ALU = mybir.AluOpType
ACT = mybir.ActivationFunctionType
DR = mybir.MatmulPerfMode.DoubleRow
np_fp8 = mybir.dt.np(FP8)

C1 = 0.2
WR, WG, WB = 0.299, 0.587, 0.114
C1T = C1 / (WG * WG)  # C1 for the (1/wg)-scaled gray values
RCP_C0 = -0.23549792
RCP_C1 = 2.0017324
K_LOSS = np.array([9, 8, 7, 6, 5, 4, 3, 2, 1], dtype=np.float64)  # K_LOSS[d]
N_CORES = 8
H = W = 1024

# acc columns: 16 for level-8 half-chunks, 4 for level-7 slabs, 2 for level-6
ACC_COLS = 22

LAST_RESULTS = None  # BassKernelResults of the most recent run (for profiling)

_CACHED_NC = None


def _ensure_ntff_hook():
    """Register the axon NTFF profile hook if the image's antenv lacks it."""
    try:
        from antenv.axon_hooks import get_axon_ntff_profile_hook

        return get_axon_ntff_profile_hook() is not None
    except ImportError:
        pass
    try:
        import types

        import antenv
        from trn_agent_boot.trn_boot import _ntff_profile_via_ctypes

        mod = types.ModuleType("antenv.axon_hooks")
        _h = {}
        mod.set_axon_ntff_profile_hook = lambda h: _h.__setitem__("h", h)
        mod.get_axon_ntff_profile_hook = lambda: _h.get("h")
        sys.modules["antenv.axon_hooks"] = mod
        antenv.axon_hooks = mod
        hook = _ntff_profile_via_ctypes("/opt/axon/libaxon_pjrt.so")
        mod.set_axon_ntff_profile_hook(hook)
        from concourse import bass_utils as _bu

        _bu.upload_artifacts = lambda tmpdir: tmpdir
        return hook is not None
    except Exception as e:  # pragma: no cover - profiling-only path
        print(f"ntff hook setup failed: {type(e).__name__}: {e}")
        return False


def _weight_matrices():
    """wdr [5,128,256] fp8: DoubleRow k-stacked [128, k=2, 128] weights
    (+I,+I), (-I,-I), (+I,0), (0,+I), (+I,-I).
    wpr [4,128,128] f32(r): Pa, Pb row-pair pooling, +I, -I."""
    eye = np.eye(128, dtype=np.float32)
    zero = np.zeros((128, 128), dtype=np.float32)
    def k2(a, b):
        return np.stack([a, b], axis=1).reshape(128, 256)
    wdr = np.stack(
        [k2(eye, eye), k2(-eye, -eye), k2(eye, zero), k2(zero, eye), k2(eye, -eye)]
    ).astype(np_fp8)
    wpr = np.zeros((4, 128, 128), dtype=np.float32)
    for j in range(64):
        wpr[0, 2 * j, j] = 1.0       # Pa: even chunk row pairs -> part 0..63
        wpr[0, 2 * j + 1, j] = 1.0
        wpr[1, 2 * j, 64 + j] = 1.0  # Pb: odd chunk row pairs -> part 64..127
        wpr[1, 2 * j + 1, 64 + j] = 1.0
    wpr[2] = eye
    wpr[3] = -eye
    return wdr, wpr


def _build_nc():
    nc = bacc.Bacc("TRN2", target_bir_lowering=False, debug=False)

    rgb_d = nc.declare_dram_parameter("rgbxy", [8, 128, 6144], FP8, isOutput=False)
    wdr_d = nc.declare_dram_parameter("wdr", [5, 128, 256], FP8, isOutput=False)
    wpr_d = nc.declare_dram_parameter("wpr", [4, 128, 128], F32R, isOutput=False)
    acc_d = nc.declare_dram_parameter("acc", [128, ACC_COLS], F32, isOutput=True)
    s5t5_d = nc.declare_dram_parameter("s5t5", [128, 256], F32, isOutput=True)

    with tile.TileContext(nc) as tc:
        with (
            tc.tile_pool(name="singles", bufs=1) as singles,
            tc.tile_pool(name="rgb", bufs=4) as rgb_pool,
            tc.tile_pool(name="gray", bufs=4) as gray_pool,
            tc.tile_pool(name="sd", bufs=2) as sd_pool,
            tc.tile_pool(name="pg", bufs=2, space="PSUM") as pg_pool,
            tc.tile_pool(name="pv", bufs=2, space="PSUM") as pv_pool,
            tc.tile_pool(name="pp", bufs=2, space="PSUM") as pp_pool,
        ):
            # --- weights (scalar queue: own HWDGE, parallel with inputs) ---
            wdr_t = [
                singles.tile([128, 256], FP8, tag=f"wdr{i}", name=f"wdr{i}")
                for i in range(5)
            ]
            for i in range(5):
                nc.scalar.dma_start(wdr_t[i][:], wdr_d[i])
            w_pp, w_nn, w_p0, w_0p, w_pn = [
                t[:].rearrange("p (k m) -> p k m", k=2) for t in wdr_t
            ]
            wpr_t = [
                singles.tile([128, 128], F32R, tag=f"wpr{i}", name=f"wpr{i}")
                for i in range(4)
            ]
            for i in range(4):
                nc.scalar.dma_start(wpr_t[i][:], wpr_d[i])
            pa, pb, ip, im = [t[:] for t in wpr_t]

            # --- inputs: one fp8 DMA per 128-row chunk ---
            rgb = [
                rgb_pool.tile([128, 6144], FP8, tag="rgb", name=f"rgb{j}")
                for j in range(8)
            ]
            for j in range(8):
                nc.sync.dma_start(rgb[j][:], rgb_d[j])

            acc = singles.tile([128, ACC_COLS], F32)
            s5t5 = singles.tile([128, 256], F32)
            s7all = singles.tile([128, 2048], F32, tag="s7all")
            t7all = singles.tile([128, 2048], F32, tag="t7all")
            s6all = singles.tile([128, 512], F32, tag="s6all")
            t6all = singles.tile([128, 512], F32, tag="t6all")
            dead = singles.tile([128, 2048], F32, tag="dead")

            def gray_chunk(j, col0):
                """Chunk j: s,t gray -> g8 [128, 2048] f32 (x|y) via DR
                matmuls + scalar evac; diff -> 2 PSUM slabs; then the two
                vector passes (den + rcpmul with psum diff) per 512-slab.
                acc cols col0, col0+1."""
                v = rgb[j][:].rearrange("p (xy c n) -> p xy c n", xy=2, c=3)
                g = gray_pool.tile([128, 2048], F32, tag="g8")
                pss = pg_pool.tile([128, 1024], F32, tag="pg", name="pss")
                pst = pg_pool.tile([128, 1024], F32, tag="pg", name="pst")
                slabs = (slice(0, 512), slice(512, 1024))
                # s,t gray: weight-major, each weight loaded once
                for sl in slabs:
                    nc.tensor.matmul(
                        pss[:, sl], w_pp, v[:, 0, 0:2, sl],
                        start=True, stop=False, perf_mode=DR,
                    )
                for sl in slabs:
                    nc.tensor.matmul(
                        pst[:, sl], w_pp, v[:, 1, 0:2, sl],
                        start=True, stop=False, perf_mode=DR,
                    )
                for sl in slabs:
                    nc.tensor.matmul(
                        pss[:, sl], w_p0, v[:, :, 2, sl],
                        start=False, stop=True, perf_mode=DR,
                    )
                for sl in slabs:
                    nc.tensor.matmul(
                        pst[:, sl], w_0p, v[:, :, 2, sl],
                        start=False, stop=True, perf_mode=DR,
                    )
                nc.scalar.activation(g[:, 0:1024].bitcast(F32R), pss[:], ACT.Copy)
                nc.scalar.activation(g[:, 1024:2048].bitcast(F32R), pst[:], ACT.Copy)
                # diff = gray_x - gray_y, one PSUM slab at a time
                den = sd_pool.tile([128, 1024], F32, tag="den8")
                nc.vector._custom_dve(
                    DEN_SSIM, out=den[:], in0=g[:, 0:1024], in1=g[:, 1024:2048],
                    s0=C1T,
                )
                for si, sl in enumerate(slabs):
                    pvv = pv_pool.tile([128, 512], F32, tag="pv", name="pvv")
                    nc.tensor.matmul(
                        pvv[:], w_pp, v[:, 0, 0:2, sl],
                        start=True, stop=False, perf_mode=DR,
                    )
                    nc.tensor.matmul(
                        pvv[:], w_nn, v[:, 1, 0:2, sl],
                        start=False, stop=False, perf_mode=DR,
                    )
                    nc.tensor.matmul(
                        pvv[:], w_pn, v[:, :, 2, sl],
                        start=False, stop=True, perf_mode=DR,
                    )
                    nc.vector._custom_dve(
                        RCPMUL,
                        out=dead[:, 0:512],
                        in0=den[:, sl],
                        in1=pvv[:],
                        s0=RCP_C0,
                        s1=RCP_C1,
                        accum_out=acc[:, col0 + si : col0 + si + 1],
                    )
                return g

            def pool_pair(even_ap, odd_ap, fd, out_ap, round_f32r=True):
                """2x2 sum-pool two stacked [128, fd] f32 chunks ->
                out_ap [128, fd//2].  Row pairs via Pa/Pb f32r matmuls,
                column pairs via stride-2 rhs views accumulating into one
                PSUM region; scalar engine evacuates."""
                ps = pp_pool.tile([128, 512], F32, tag="pp", name="ps")
                half = fd // 2
                seq = (
                    (pa, even_ap[:, 0:fd:2]),
                    (pa, even_ap[:, 1:fd:2]),
                    (pb, odd_ap[:, 0:fd:2]),
                    (pb, odd_ap[:, 1:fd:2]),
                )
                for i, (wm, src) in enumerate(seq):
                    nc.tensor.matmul(
                        ps[:, 0:half], wm, src.bitcast(F32R),
                        start=(i == 0), stop=(i == 3),
                    )
                if round_f32r:
                    out_ap = out_ap.bitcast(F32R)
                nc.scalar.activation(out_ap, ps[:, 0:half], ACT.Copy)

            def ssim_sbuf(s_ap, t_ap, fd, col, tag):
                """SSIM on one [128, fd<=512] slab with s,t in SBUF: diff
                via +-I f32r matmuls into PSUM, den on vector, fused rcpmul."""
                den = sd_pool.tile([128, fd], F32, tag=f"den{tag}", name="den")
                nc.vector._custom_dve(
                    DEN_SSIM, out=den[:], in0=s_ap, in1=t_ap, s0=C1T
                )
                pvv = pv_pool.tile([128, 512], F32, tag="pv", name="pvl")
                nc.tensor.matmul(
                    pvv[:, 0:fd], ip, s_ap.bitcast(F32R), start=True, stop=False
                )
                nc.tensor.matmul(
                    pvv[:, 0:fd], im, t_ap.bitcast(F32R), start=False, stop=True
                )
                nc.vector._custom_dve(
                    RCPMUL,
                    out=dead[:, 0:fd],
                    in0=den[:],
                    in1=pvv[:, 0:fd],
                    s0=RCP_C0,
                    s1=RCP_C1,
                    accum_out=acc[:, col : col + 1],
                )

            def colpool(src_ap, fd, tag):
                """GpSimd: add adjacent column pairs, [128,fd] -> [128,fd/2]."""
                c = sd_pool.tile([128, fd // 2], F32R, tag=f"cp{tag}", name="cp")
                nc.gpsimd.tensor_tensor(
                    c[:], src_ap[:, 0:fd:2], src_ap[:, 1:fd:2], ALU.add
                )
                return c

            def pool_pair_cf(even_ap, odd_ap, fd, out_ap, tag):
                """2x2 sum-pool, colpool-first variant: column pairs on
                gpsimd, row pairs via 2 contiguous-rhs f32r matmuls."""
                half = fd // 2
                ce = colpool(even_ap, fd, tag + "e")
                co = colpool(odd_ap, fd, tag + "o")
                ps = pp_pool.tile([128, 512], F32, tag="pp", name="psc")
                nc.tensor.matmul(ps[:, 0:half], pa, ce[:], start=True, stop=False)
                nc.tensor.matmul(ps[:, 0:half], pb, co[:], start=False, stop=True)
                nc.scalar.activation(out_ap.bitcast(F32R), ps[:, 0:half], ACT.Copy)

            # ---- fully interleaved pyramid: per L8 chunk pair, pool to
            # L7 and immediately run the L7 slab ssim; per L7 pair, pool
            # to L6 and run its slab ssim; final L6 pool to level 5. ----
            g8 = [None] * 8
            for j in range(8):
                g8[j] = gray_chunk(j, 2 * j)
                if j % 2 != 1:
                    continue
                k = j // 2
                ge, go = g8[j - 1], g8[j]
                s7k = s7all[:, 512 * k : 512 * (k + 1)]
                t7k = t7all[:, 512 * k : 512 * (k + 1)]
                pool_pair_cf(ge[:, 0:1024], go[:, 0:1024], 1024, s7k, f"s{k % 2}")
                pool_pair_cf(ge[:, 1024:2048], go[:, 1024:2048], 1024, t7k,
                             f"t{k % 2}")
                ssim_sbuf(s7k, t7k, 512, 16 + k, "7")
                if k % 2 != 1:
                    continue
                kk = k // 2
                s6k = s6all[:, 256 * kk : 256 * (kk + 1)]
                t6k = t6all[:, 256 * kk : 256 * (kk + 1)]
                pool_pair_cf(
                    s7all[:, 1024 * kk : 1024 * kk + 512],
                    s7all[:, 1024 * kk + 512 : 1024 * (kk + 1)],
                    512, s6k, "s6",
                )
                pool_pair_cf(
                    t7all[:, 1024 * kk : 1024 * kk + 512],
                    t7all[:, 1024 * kk + 512 : 1024 * (kk + 1)],
                    512, t6k, "t6",
                )
                ssim_sbuf(s6k, t6k, 256, 20 + kk, "6")

            # ---- level 6 -> 5 pool (host handles levels 5..0) ----
            pool_pair_cf(
                s6all[:, 0:256], s6all[:, 256:512], 256, s5t5[:, 0:128], "s5"
            )
            pool_pair_cf(
                t6all[:, 0:256], t6all[:, 256:512], 256, s5t5[:, 128:256], "t5"
            )

            nc.sync.dma_start(acc_d[:], acc[:])
            nc.sync.dma_start(s5t5_d[:], s5t5[:])

    nc.compile()
    return nc


def _get_nc():
    global _CACHED_NC
    if _CACHED_NC is None:
        _CACHED_NC = _build_nc()
    return _CACHED_NC


def _host_tail(per_core):
    """Combine per-core results into the scalar loss (float64 host math)."""
    total = 0.0
    # device levels: 8 (acc cols 0..15), 7 (16..19), 6 (20..21)
    for d, cols in ((8, slice(0, 16)), (7, slice(16, 20)), (6, slice(20, 22))):
        s = sum(float(r["acc"][:, cols].astype(np.float64).sum()) for r in per_core)
        cnt = N_CORES * 16 * 4**d
        total += K_LOSS[d] * (s / cnt)
    # host levels: 5..0 on the shipped pooled images ((1/wg)-scaled values)
    s = np.stack([r["s5t5"][:, 0:128] for r in per_core]).astype(np.float64)
    t = np.stack([r["s5t5"][:, 128:256] for r in per_core]).astype(np.float64)
    for d in range(5, -1, -1):
        ratio = (s - t) ** 2 / (s * s + t * t + C1T)
        cnt = N_CORES * 16 * 4**d
        total += K_LOSS[d] * (ratio.sum() / cnt)
        if d > 0:
            b, n, _ = s.shape
            s = s.reshape(b, n // 2, 2, n // 2, 2).sum(axis=(2, 4))
            t = t.reshape(b, n // 2, 2, n // 2, 2).sum(axis=(2, 4))
    return np.float32(total)


def _pack_inputs(input, target):
    """[8,3,1024,1024] f32 x2 -> per-core [8,128,6144] fp8e4m3.
    Partition line layout [x|y][R',B',G][1024]; R,B pre-scaled by their
    grayscale weights so device mix weights are exactly +-1."""
    scale = np.array([WR / WG, WB / WG, 1.0], dtype=np.float32)[:, None, None]
    # reorder channels to (R, B, G) then scale
    xin = input[:, (0, 2, 1)] * scale
    yin = target[:, (0, 2, 1)] * scale
    out = np.empty((N_CORES, 8, 128, 2, 3, 1024), dtype=np_fp8)
    out[:, :, :, 0, :, :] = xin.reshape(8, 3, 8, 128, 1024).transpose(0, 2, 3, 1, 4)
    out[:, :, :, 1, :, :] = yin.reshape(8, 3, 8, 128, 1024).transpose(0, 2, 3, 1, 4)
    return out.reshape(N_CORES, 8, 128, 6144)


def kernel(input, target):
    global LAST_RESULTS
    input = np.ascontiguousarray(np.asarray(input, dtype=np.float32))
    target = np.ascontiguousarray(np.asarray(target, dtype=np.float32))
    assert input.shape == (N_CORES, 3, H, W), input.shape

    nc = _get_nc()
    rgbxy = _pack_inputs(input, target)
    wdr, wpr = _weight_matrices()
    in_maps = [
        {"rgbxy": rgbxy[i], "wdr": wdr, "wpr": wpr} for i in range(N_CORES)
    ]
    trace = bool(int(os.environ.get("BASS_SSIM_TRACE", "0")))
    if trace:
        trace = _ensure_ntff_hook()
    res = run_bass_kernel_spmd(nc, in_maps, list(range(N_CORES)), trace=trace)
    LAST_RESULTS = res
    return _host_tail(res.results)


# revision 33
# speedup vs baseline: 1.1591x; 1.0468x over previous
"""Trainium2 Bass kernel for nn_DividedSsimLoss.

Reference: for 8 RGB 1024x1024 image pairs, grayscale, tile 256x256,
9-level 2x2 sum-pool pyramid, loss = sum_d K[d] * (1 - mean ssim_d),
ssim = (2st + C1) / (s^2 + t^2 + C1), i.e. 1-ssim = (s-t)^2/(s^2+t^2+C1).

v4 design (per core = one image pair, pure data parallelism):
  * Host ships the 6 channel planes as fp8e4m3, with R and B pre-scaled
    by their grayscale weights (wr/wg, wb/wg) so that every device-side
    channel-mix weight is exactly +-1.0 in fp8.  Layout per partition
    line: [x|y][R',B',G][1024] -> one [128, 6144] fp8 DMA per 128-row
    chunk (6 KiB contiguous DRAM per partition line).  6 MiB per core.
  * Grayscale (scaled 1/wg) runs on the tensor engine with fp8
    DoubleRow matmuls: one DR matmul contracts the stacked [R'; B']
    2-k-tile pair at 0.5 cyc/row, a second DR matmul adds G via a
    [I; 0] / [0; I] weight pair.  PSUM accumulates in f32; the scalar
    engine evacuates to SBUF as f32r-rounded f32.
  * diff = gray_x - gray_y is ALSO computed on the tensor engine
    ([I;I] / [-I;-I] / [I;-I] DR matmuls into separate PSUM slabs),
    exactly consistent with s - t.
  * Per level the vector engine runs only 2 custom DVE passes:
    DEN = s^2+t^2+C1 (from SBUF) and the fused
    RCPMUL: accum += diff^2 * recip_approx(den), with diff read
    directly from PSUM (one PSUM operand is allowed).
  * 2x2 pooling: row pairs via Pa/Pb f32r matmuls with stride-2 rhs
    views (column pairs fold into the same PSUM accumulation), scalar
    engine evacuates.  Levels 7/6 pyramid diffs via +-I f32r matmuls.
  * Device covers levels 8,7,6 + pooled level-5 images; host does
    levels 5..0 in f64.
"""

import os
import sys

import numpy as np

for _p in ("/opt/trn_rl_repo",):
    if _p not in sys.path:
        sys.path.insert(0, _p)

import concourse.bacc as bacc
import concourse.bass as bass
import concourse.mybir as mybir
import concourse.tile as tile
from concourse.bass_utils import run_bass_kernel_spmd

from ml_dtypes import bfloat16 as np_bf16


def _register_dve_ops():
    """Register kernel-specific custom DVE ops (idempotent).

    DEN_SSIM:  out = in0^2 + in1^2 + s0
    RCPMUL:    out = in1^2 * y1(in0),  accum = sum(out)
               y1 = one-NR reciprocal approx of in0 (bitwise-not seed)
    """
    import concourse.dve_ops as dve_ops
    from concourse.dve_ops import DveOp
    from concourse.dve_spec import (
        C0,
        C1,
        AluOp,
        Bin,
        Spec,
        Src0,
        Src1,
        _has_src1,
        lower,
        sq,
    )
    from concourse.dve_uop import DveOpSpec
    from operator import add as _add

    def _sha_for(name, spec):
        shas = {}
        for ver in ("v3",):
            row = dve_ops._SUB_OPCODE_FOR_NAME[name]
            s = DveOpSpec(
                name=name, opcode=row, uops=lower(spec, ver=ver),
                rd1_en=_has_src1(spec),
            )
            shas[ver] = s.sha(ver)
        return shas

    def _register(name, spec):
        if name in dve_ops._SUB_OPCODE_FOR_NAME:
            return next(op for op in dve_ops.OPS if op.name == name)
        row = dve_ops._CUSTOM_DVE_ROW_BASE + len(dve_ops.OPS)
        assert row < 0x20, "custom-DVE row field overflow"
        dve_ops._SUB_OPCODE_FOR_NAME[name] = row
        op = DveOp(name, spec, subdim=False, uops_sha=_sha_for(name, spec))
        dve_ops.OPS.append(op)
        dve_ops.CUSTOM_DVE_SPECS[name] = spec
        return op

    den_spec = Spec(
        body=sq(Src0) + sq(Src1) + C0,
        reference=lambda in0, in1, s0, s1, imm2: (
            in0.astype(np.float32) ** 2 + in1.astype(np.float32) ** 2 + s0
        ),
    )

    # reciprocal seed: x * bitcast(~x) lands in [-4.5, -4]; one Chebyshev
    # scale + one NR pass (same constants as RECIPROCAL_APPROX_FAST).
    _nx = Bin(AluOp.BITWISE_NOT, Src0, Src0)
    _y0 = _nx * C0
    _y1 = _y0 * (C1 - Src0 * _y0)

    def _ref_rcpmul(in0, in1, c0, c1, c2):
        not_x = (~in0.astype(np.float32).view(np.int32)).view(np.float32)
        y0 = not_x * c0
        y1 = y0 * (c1 - in0.astype(np.float32) * y0)
        return in1.astype(np.float32) ** 2 * y1

    rcpmul_spec = Spec(
        body=sq(Src1) * _y1,
        accum=_add,
        reference=dve_ops._ref_body_sum(_ref_rcpmul),
    )

    return (
        _register("DEN_SSIM_ANT", den_spec),
        _register("RCPMUL_SSIM_ANT", rcpmul_spec),
    )


DEN_SSIM, RCPMUL = _register_dve_ops()

F32 = mybir.dt.float32
F32R = mybir.dt.float32r
BF16 = mybir.dt.bfloat16
FP8 = mybir.dt.float8e4
ALU = mybir.AluOpType
ACT = mybir.ActivationFunctionType
DR = mybir.MatmulPerfMode.DoubleRow
np_fp8 = mybir.dt.np(FP8)

C1 = 0.2
WR, WG, WB = 0.299, 0.587, 0.114
C1T = C1 / (WG * WG)  # C1 for the (1/wg)-scaled gray values
RCP_C0 = -0.23549792
RCP_C1 = 2.0017324
K_LOSS = np.array([9, 8, 7, 6, 5, 4, 3, 2, 1], dtype=np.float64)  # K_LOSS[d]
N_CORES = 8
H = W = 1024

# acc columns: 16 for level-8 half-chunks, 4 for level-7 slabs, 2 for level-6
ACC_COLS = 22

LAST_RESULTS = None  # BassKernelResults of the most recent run (for profiling)

_CACHED_NC = None


def _ensure_ntff_hook():
    """Register the axon NTFF profile hook if the image's antenv lacks it."""
    try:
        from antenv.axon_hooks import get_axon_ntff_profile_hook

        return get_axon_ntff_profile_hook() is not None
    except ImportError:
        pass
    try:
        import types

        import antenv
        from trn_agent_boot.trn_boot import _ntff_profile_via_ctypes

        mod = types.ModuleType("antenv.axon_hooks")
        _h = {}
        mod.set_axon_ntff_profile_hook = lambda h: _h.__setitem__("h", h)
        mod.get_axon_ntff_profile_hook = lambda: _h.get("h")
        sys.modules["antenv.axon_hooks"] = mod
        antenv.axon_hooks = mod
        hook = _ntff_profile_via_ctypes("/opt/axon/libaxon_pjrt.so")
        mod.set_axon_ntff_profile_hook(hook)
        from concourse import bass_utils as _bu

        _bu.upload_artifacts = lambda tmpdir: tmpdir
        return hook is not None
    except Exception as e:  # pragma: no cover - profiling-only path
        print(f"ntff hook setup failed: {type(e).__name__}: {e}")
        return False


def _weight_matrices():
    """wdr [5,128,256] fp8: DoubleRow k-stacked [128, k=2, 128] weights
    (+I,+I), (-I,-I), (+I,0), (0,+I), (+I,-I).
    wpr [4,128,128] f32(r): Pa, Pb row-pair pooling, +I, -I."""
    eye = np.eye(128, dtype=np.float32)
    zero = np.zeros((128, 128), dtype=np.float32)
    def k2(a, b):
        return np.stack([a, b], axis=1).reshape(128, 256)
    wdr = np.stack(
        [k2(eye, eye), k2(-eye, -eye), k2(eye, zero), k2(zero, eye), k2(eye, -eye)]
    ).astype(np_fp8)
    wpr = np.zeros((4, 128, 128), dtype=np.float32)
    for j in range(64):
        wpr[0, 2 * j, j] = 1.0       # Pa: even chunk row pairs -> part 0..63
        wpr[0, 2 * j + 1, j] = 1.0
        wpr[1, 2 * j, 64 + j] = 1.0  # Pb: odd chunk row pairs -> part 64..127
        wpr[1, 2 * j + 1, 64 + j] = 1.0
    wpr[2] = eye
    wpr[3] = -eye
    return wdr, wpr


def _build_nc():
    nc = bacc.Bacc("TRN2", target_bir_lowering=False, debug=False)

    rgb_d = nc.declare_dram_parameter("rgbxy", [8, 128, 6144], FP8, isOutput=False)
    wdr_d = nc.declare_dram_parameter("wdr", [5, 128, 256], FP8, isOutput=False)
    wpr_d = nc.declare_dram_parameter("wpr", [4, 128, 128], F32R, isOutput=False)
    acc_d = nc.declare_dram_parameter("acc", [128, ACC_COLS], F32, isOutput=True)
    s5t5_d = nc.declare_dram_parameter("s5t5", [128, 256], F32, isOutput=True)

    with tile.TileContext(nc) as tc:
        with (
            tc.tile_pool(name="singles", bufs=1) as singles,
            tc.tile_pool(name="rgb", bufs=4) as rgb_pool,
            tc.tile_pool(name="gray", bufs=4) as gray_pool,
            tc.tile_pool(name="sd", bufs=2) as sd_pool,
            tc.tile_pool(name="pg", bufs=2, space="PSUM") as pg_pool,
            tc.tile_pool(name="pv", bufs=2, space="PSUM") as pv_pool,
            tc.tile_pool(name="pp", bufs=2, space="PSUM") as pp_pool,
        ):
            # --- weights (scalar queue: own HWDGE, parallel with inputs) ---
            wdr_t = [
                singles.tile([128, 256], FP8, tag=f"wdr{i}", name=f"wdr{i}")
                for i in range(5)
            ]
            for i in range(5):
                nc.scalar.dma_start(wdr_t[i][:], wdr_d[i])
            w_pp, w_nn, w_p0, w_0p, w_pn = [
                t[:].rearrange("p (k m) -> p k m", k=2) for t in wdr_t
            ]
            wpr_t = [
                singles.tile([128, 128], F32R, tag=f"wpr{i}", name=f"wpr{i}")
                for i in range(4)
            ]
            for i in range(4):
                nc.scalar.dma_start(wpr_t[i][:], wpr_d[i])
            pa, pb, ip, im = [t[:] for t in wpr_t]

            # --- inputs: one fp8 DMA per 128-row chunk ---
            rgb = [
                rgb_pool.tile([128, 6144], FP8, tag="rgb", name=f"rgb{j}")
                for j in range(8)
            ]
            for j in range(8):
                nc.sync.dma_start(rgb[j][:], rgb_d[j])

            acc = singles.tile([128, ACC_COLS], F32)
            s5t5 = singles.tile([128, 256], F32)
            s7all = singles.tile([128, 2048], F32, tag="s7all")
            t7all = singles.tile([128, 2048], F32, tag="t7all")
            s6all = singles.tile([128, 512], F32, tag="s6all")
            t6all = singles.tile([128, 512], F32, tag="t6all")
            dead = singles.tile([128, 2048], F32, tag="dead")

            def gray_chunk(j, col0):
                """Chunk j: s,t gray -> g8 [128, 2048] f32 (x|y) via DR
                matmuls + scalar evac; diff -> 2 PSUM slabs; then the two
                vector passes (den + rcpmul with psum diff) per 512-slab.
                acc cols col0, col0+1."""
                v = rgb[j][:].rearrange("p (xy c n) -> p xy c n", xy=2, c=3)
                g = gray_pool.tile([128, 2048], F32, tag="g8")
                pss = pg_pool.tile([128, 1024], F32, tag="pg", name="pss")
                pst = pg_pool.tile([128, 1024], F32, tag="pg", name="pst")
                slabs = (slice(0, 512), slice(512, 1024))
                # s,t gray: weight-major, each weight loaded once
                for sl in slabs:
                    nc.tensor.matmul(
                        pss[:, sl], w_pp, v[:, 0, 0:2, sl],
                        start=True, stop=False, perf_mode=DR,
                    )
                for sl in slabs:
                    nc.tensor.matmul(
                        pst[:, sl], w_pp, v[:, 1, 0:2, sl],
                        start=True, stop=False, perf_mode=DR,
                    )
                for sl in slabs:
                    nc.tensor.matmul(
                        pss[:, sl], w_p0, v[:, :, 2, sl],
                        start=False, stop=True, perf_mode=DR,
                    )
                for sl in slabs:
                    nc.tensor.matmul(
                        pst[:, sl], w_0p, v[:, :, 2, sl],
                        start=False, stop=True, perf_mode=DR,
                    )
                nc.scalar.activation(g[:, 0:1024].bitcast(F32R), pss[:], ACT.Copy)
                nc.scalar.activation(g[:, 1024:2048].bitcast(F32R), pst[:], ACT.Copy)
                # den reads t from PSUM: avoids the shared SBUF read port,
                # so gpsimd colpools don't contend with it
                den = sd_pool.tile([128, 1024], F32, tag="den8")
                nc.vector._custom_dve(
                    DEN_SSIM, out=den[:], in0=g[:, 0:1024], in1=pst[:],
                    s0=C1T,
                )
                for si, sl in enumerate(slabs):
                    pvv = pv_pool.tile([128, 512], F32, tag="pv", name="pvv")
                    nc.tensor.matmul(
                        pvv[:], w_pp, v[:, 0, 0:2, sl],
                        start=True, stop=False, perf_mode=DR,
                    )
                    nc.tensor.matmul(
                        pvv[:], w_nn, v[:, 1, 0:2, sl],
                        start=False, stop=False, perf_mode=DR,
                    )
                    nc.tensor.matmul(
                        pvv[:], w_pn, v[:, :, 2, sl],
                        start=False, stop=True, perf_mode=DR,
                    )
                    nc.vector._custom_dve(
                        RCPMUL,
                        out=dead[:, 0:512],
                        in0=den[:, sl],
                        in1=pvv[:],
                        s0=RCP_C0,
                        s1=RCP_C1,
                        accum_out=acc[:, col0 + si : col0 + si + 1],
                    )
                return g

            def pool_pair(even_ap, odd_ap, fd, out_ap, round_f32r=True):
                """2x2 sum-pool two stacked [128, fd] f32 chunks ->
                out_ap [128, fd//2].  Row pairs via Pa/Pb f32r matmuls,
                column pairs via stride-2 rhs views accumulating into one
                PSUM region; scalar engine evacuates."""
                ps = pp_pool.tile([128, 512], F32, tag="pp", name="ps")
                half = fd // 2
                seq = (
                    (pa, even_ap[:, 0:fd:2]),
                    (pa, even_ap[:, 1:fd:2]),
                    (pb, odd_ap[:, 0:fd:2]),
                    (pb, odd_ap[:, 1:fd:2]),
                )
                for i, (wm, src) in enumerate(seq):
                    nc.tensor.matmul(
                        ps[:, 0:half], wm, src.bitcast(F32R),
                        start=(i == 0), stop=(i == 3),
                    )
                if round_f32r:
                    out_ap = out_ap.bitcast(F32R)
                nc.scalar.activation(out_ap, ps[:, 0:half], ACT.Copy)

            def ssim_sbuf(s_ap, t_ap, fd, col, tag):
                """SSIM on one [128, fd<=512] slab with s,t in SBUF: diff
                via +-I f32r matmuls into PSUM, den on vector, fused rcpmul."""
                den = sd_pool.tile([128, fd], F32, tag=f"den{tag}", name="den")
                nc.vector._custom_dve(
                    DEN_SSIM, out=den[:], in0=s_ap, in1=t_ap, s0=C1T
                )
                pvv = pv_pool.tile([128, 512], F32, tag="pv", name="pvl")
                nc.tensor.matmul(
                    pvv[:, 0:fd], ip, s_ap.bitcast(F32R), start=True, stop=False
                )
                nc.tensor.matmul(
                    pvv[:, 0:fd], im, t_ap.bitcast(F32R), start=False, stop=True
                )
                nc.vector._custom_dve(
                    RCPMUL,
                    out=dead[:, 0:fd],
                    in0=den[:],
                    in1=pvv[:, 0:fd],
                    s0=RCP_C0,
                    s1=RCP_C1,
                    accum_out=acc[:, col : col + 1],
                )

            def colpool(src_ap, fd, tag):
                """GpSimd: add adjacent column pairs, [128,fd] -> [128,fd/2]."""
                c = sd_pool.tile([128, fd // 2], F32R, tag=f"cp{tag}", name="cp")
                nc.gpsimd.tensor_tensor(
                    c[:], src_ap[:, 0:fd:2], src_ap[:, 1:fd:2], ALU.add
                )
                return c

            def pool_pair_cf(even_ap, odd_ap, fd, out_ap, tag):
                """2x2 sum-pool, colpool-first variant: column pairs on
                gpsimd, row pairs via 2 contiguous-rhs f32r matmuls."""
                half = fd // 2
                ce = colpool(even_ap, fd, tag + "e")
                co = colpool(odd_ap, fd, tag + "o")
                ps = pp_pool.tile([128, 512], F32, tag="pp", name="psc")
                nc.tensor.matmul(ps[:, 0:half], pa, ce[:], start=True, stop=False)
                nc.tensor.matmul(ps[:, 0:half], pb, co[:], start=False, stop=True)
                nc.scalar.activation(out_ap.bitcast(F32R), ps[:, 0:half], ACT.Copy)

            # ---- fully interleaved pyramid: per L8 chunk pair, pool to
            # L7 and immediately run the L7 slab ssim; per L7 pair, pool
            # to L6 and run its slab ssim; final L6 pool to level 5. ----
            g8 = [None] * 8
            for j in range(8):
                g8[j] = gray_chunk(j, 2 * j)
                if j % 2 != 1:
                    continue
                k = j // 2
                ge, go = g8[j - 1], g8[j]
                s7k = s7all[:, 512 * k : 512 * (k + 1)]
                t7k = t7all[:, 512 * k : 512 * (k + 1)]
                pool_pair_cf(ge[:, 0:1024], go[:, 0:1024], 1024, s7k, f"s{k % 2}")
                pool_pair_cf(ge[:, 1024:2048], go[:, 1024:2048], 1024, t7k,
                             f"t{k % 2}")
                ssim_sbuf(s7k, t7k, 512, 16 + k, "7")
                if k % 2 != 1:
                    continue
                kk = k // 2
                s6k = s6all[:, 256 * kk : 256 * (kk + 1)]
                t6k = t6all[:, 256 * kk : 256 * (kk + 1)]
                pool_pair_cf(
                    s7all[:, 1024 * kk : 1024 * kk + 512],
                    s7all[:, 1024 * kk + 512 : 1024 * (kk + 1)],
                    512, s6k, "s6",
                )
                pool_pair_cf(
                    t7all[:, 1024 * kk : 1024 * kk + 512],
                    t7all[:, 1024 * kk + 512 : 1024 * (kk + 1)],
                    512, t6k, "t6",
                )
                ssim_sbuf(s6k, t6k, 256, 20 + kk, "6")

            # ---- level 6 -> 5 pool (host handles levels 5..0) ----
            pool_pair_cf(
                s6all[:, 0:256], s6all[:, 256:512], 256, s5t5[:, 0:128], "s5"
            )
            pool_pair_cf(
                t6all[:, 0:256], t6all[:, 256:512], 256, s5t5[:, 128:256], "t5"
            )

            nc.sync.dma_start(acc_d[:], acc[:])
            nc.sync.dma_start(s5t5_d[:], s5t5[:])

    nc.compile()
    return nc


def _get_nc():
    global _CACHED_NC
    if _CACHED_NC is None:
        _CACHED_NC = _build_nc()
    return _CACHED_NC


def _host_tail(per_core):
    """Combine per-core results into the scalar loss (float64 host math)."""
    total = 0.0
    # device levels: 8 (acc cols 0..15), 7 (16..19), 6 (20..21)
    for d, cols in ((8, slice(0, 16)), (7, slice(16, 20)), (6, slice(20, 22))):
        s = sum(float(r["acc"][:, cols].astype(np.float64).sum()) for r in per_core)
        cnt = N_CORES * 16 * 4**d
        total += K_LOSS[d] * (s / cnt)
    # host levels: 5..0 on the shipped pooled images ((1/wg)-scaled values)
    s = np.stack([r["s5t5"][:, 0:128] for r in per_core]).astype(np.float64)
    t = np.stack([r["s5t5"][:, 128:256] for r in per_core]).astype(np.float64)
    for d in range(5, -1, -1):
        ratio = (s - t) ** 2 / (s * s + t * t + C1T)
        cnt = N_CORES * 16 * 4**d
        total += K_LOSS[d] * (ratio.sum() / cnt)
        if d > 0:
            b, n, _ = s.shape
            s = s.reshape(b, n // 2, 2, n // 2, 2).sum(axis=(2, 4))
            t = t.reshape(b, n // 2, 2, n // 2, 2).sum(axis=(2, 4))
    return np.float32(total)


def _pack_inputs(input, target):
    """[8,3,1024,1024] f32 x2 -> per-core [8,128,6144] fp8e4m3.
    Partition line layout [x|y][R',B',G][1024]; R,B pre-scaled by their
    grayscale weights so device mix weights are exactly +-1."""
    scale = np.array([WR / WG, WB / WG, 1.0], dtype=np.float32)[:, None, None]
    # reorder channels to (R, B, G) then scale
    xin = input[:, (0, 2, 1)] * scale
    yin = target[:, (0, 2, 1)] * scale
    out = np.empty((N_CORES, 8, 128, 2, 3, 1024), dtype=np_fp8)
    out[:, :, :, 0, :, :] = xin.reshape(8, 3, 8, 128, 1024).transpose(0, 2, 3, 1, 4)
    out[:, :, :, 1, :, :] = yin.reshape(8, 3, 8, 128, 1024).transpose(0, 2, 3, 1, 4)
    return out.reshape(N_CORES, 8, 128, 6144)


def kernel(input, target):
    global LAST_RESULTS
    input = np.ascontiguousarray(np.asarray(input, dtype=np.float32))
    target = np.ascontiguousarray(np.asarray(target, dtype=np.float32))
    assert input.shape == (N_CORES, 3, H, W), input.shape

    nc = _get_nc()
    rgbxy = _pack_inputs(input, target)
    wdr, wpr = _weight_matrices()
    in_maps = [
        {"rgbxy": rgbxy[i], "wdr": wdr, "wpr": wpr} for i in range(N_CORES)
    ]
    trace = bool(int(os.environ.get("BASS_SSIM_TRACE", "0")))
    if trace:
        trace = _ensure_ntff_hook()
    res = run_bass_kernel_spmd(nc, in_maps, list(range(N_CORES)), trace=trace)
    LAST_RESULTS = res
    return _host_tail(res.results)
